# revision 27
# baseline (speedup 1.0000x reference)
"""Trainium2 Bass kernel for DistanceGatedScoringFunction (fp8 version).

Computation (per row n of the batch):
  gl     = gate_input @ Wg + bg                       [L]
  logits = -(||gl||^2 - 2 gl @ centers.T + ||c||^2)   [E]
  logits = relu(logits @ Wgm1 + bgm1) @ Wgm2 + bgm2   [E]
  probs  = softmax(logits + gumbel)                   [E]
  eo_e   = (relu(relu(x @ We1_e + be1_e) @ We2_e + be2_e)) @ We3_e + be3_e
  out    = sigmoid(sum_e eo_e * probs_e)              [1]

Strategy: data-parallel over 8 NeuronCores (shard N), replicate params.
Feature-major on-chip layout (features on partitions, tokens on free dim).

Expert path runs in fp8 (e4m3) with MatmulPerfMode.DoubleRow: one matmul
instruction contracts 256 (2 k-tiles of 128) at 1 cycle/output-row —
2x the fp32r rate per contraction.  Validated end-to-end rel err ~1.2e-2
(gate 2e-2): error is dominated by fp8 weight quantization.

Gating stays fp32r for precision.  The distance computation is folded
into the gating-MLP first layer (both are linear):
  logits1 = [gl; gl^2] @ A + beff,  A built host-side from centers/Wgm1.
The gumbel add rides the PE as an identity-matmul accumulation into the
mlp2 PSUM.  Softmax is computed unnormalized (exp of centered logits,
final division in a batched post-pass).

relu+bias+downcast ops are split between ACT (Scalar) and DVE (Vector);
the gating square runs on the otherwise idle Pool (GpSimd) engine
(Pool cannot read PSUM, so it only gets SBUF-source work).
"""

import numpy as np

N, D, H, E, L = 100000, 256, 256, 8, 64
M_CORES = 8
NC_N = N // M_CORES     # rows per core
F = 500                 # token tile (moving free dim)


def _build_nc(nc_n, f):
    from contextlib import ExitStack

    import concourse.bacc as bacc
    import concourse.mybir as mybir
    import concourse.tile as tile

    fp32 = mybir.dt.float32
    fr = mybir.dt.float32r
    f8 = mybir.dt.float8e4
    AF = mybir.ActivationFunctionType
    OP = mybir.AluOpType
    PM = mybir.MatmulPerfMode.DoubleRow
    t_tiles = nc_n // f
    assert t_tiles * f == nc_n
    PP = 100
    PJ = nc_n // PP
    assert PP * PJ == nc_n

    nc = bacc.Bacc("TRN2", target_bir_lowering=False, debug=False)

    # ---- DRAM I/O ----
    xs_d = nc.dram_tensor("xs", [D, nc_n], f8, kind="ExternalInput")
    xg_d = nc.dram_tensor("xg", [D, nc_n], fr, kind="ExternalInput")
    gm_d = nc.dram_tensor("gm", [E, nc_n], fr, kind="ExternalInput")
    we1_d = nc.dram_tensor("we1", [128, 2 * E, 2, 128], f8, kind="ExternalInput")
    we2_d = nc.dram_tensor("we2", [128, 2 * E, 2, 128], f8, kind="ExternalInput")
    we3_d = nc.dram_tensor("we3", [128, E, 2, 16], f8, kind="ExternalInput")
    wg_d = nc.dram_tensor("wg", [128, 2, L], fr, kind="ExternalInput")
    amat_d = nc.dram_tensor("amat", [128, 2, 128], fr, kind="ExternalInput")
    wgm2_d = nc.dram_tensor("wgm2", [128, 2, E], fr, kind="ExternalInput")
    i8_d = nc.dram_tensor("i8", [E, E], fr, kind="ExternalInput")
    dnw_d = nc.dram_tensor("dnw", [E, 4], fr, kind="ExternalInput")
    b128_d = nc.dram_tensor("b128", [128, 34], fp32, kind="ExternalInput")
    csm_d = nc.dram_tensor("csm", [L, 4], fp32, kind="ExternalInput")
    out_d = nc.dram_tensor("out", [nc_n], fp32, kind="ExternalOutput")
    scr_d = nc.dram_tensor("scr", [2, nc_n], fp32)  # den/num bounce

    xs_r = xs_d.ap().rearrange("(c p) n -> p c n", p=128)
    xg_r = xg_d.ap().rearrange("(c p) n -> p c n", p=128)

    with tile.TileContext(nc) as tc, ExitStack() as ctx:
        cw = ctx.enter_context(tc.tile_pool(name="cw", bufs=1))
        xin = ctx.enter_context(tc.tile_pool(name="xin", bufs=4))
        wk = ctx.enter_context(tc.tile_pool(name="wk", bufs=3))
        hp = ctx.enter_context(tc.tile_pool(name="hp", bufs=4))
        pbig = ctx.enter_context(tc.tile_pool(name="pbig", bufs=5, space="PSUM"))
        pmid = ctx.enter_context(tc.tile_pool(name="pmid", bufs=2, space="PSUM"))
        ppeo = ctx.enter_context(tc.tile_pool(name="ppeo", bufs=1, space="PSUM"))

        # ---- constants into SBUF (one DMA each) ----
        we1_s = cw.tile([128, 2 * E, 2, 128], f8)
        nc.sync.dma_start(out=we1_s, in_=we1_d.ap())
        we2_s = cw.tile([128, 2 * E, 2, 128], f8)
        nc.sync.dma_start(out=we2_s, in_=we2_d.ap())
        we3_s = cw.tile([128, E, 2, 16], f8)
        nc.sync.dma_start(out=we3_s, in_=we3_d.ap())
        wg_s = cw.tile([128, 2, L], fr)
        nc.sync.dma_start(out=wg_s, in_=wg_d.ap())
        amat_s = cw.tile([128, 2, 128], fr)
        nc.sync.dma_start(out=amat_s, in_=amat_d.ap())
        wgm2_s = cw.tile([128, 2, E], fr)
        nc.sync.dma_start(out=wgm2_s, in_=wgm2_d.ap())
        i8_s = cw.tile([E, E], fr)
        nc.sync.dma_start(out=i8_s, in_=i8_d.ap())
        dnw_s = cw.tile([E, 4], fr)
        nc.sync.dma_start(out=dnw_s, in_=dnw_d.ap())
        b128_s = cw.tile([128, 34], fp32)
        nc.sync.dma_start(out=b128_s, in_=b128_d.ap())
        csm_s = cw.tile([L, 4], fp32)
        nc.sync.dma_start(out=csm_s, in_=csm_d.ap())

        bg_b = csm_s[:, 0:1]            # [64,1]
        bgm2_b = csm_s[0:E, 1:2]        # [8,1]
        be3_b = csm_s[0:E, 2:3]         # [8,1]

        pending_tail = None
        for t in range(t_tiles):
            n0 = t * f
            # ---- input tiles ----
            xs_t = xin.tile([128, 2, f], f8, tag="xs", name="xs_t")
            nc.sync.dma_start(out=xs_t, in_=xs_r[:, :, n0 : n0 + f])
            xg_t = xin.tile([128, 2, f], fr, tag="xg", name="xg_t")
            nc.sync.dma_start(out=xg_t, in_=xg_r[:, :, n0 : n0 + f])
            gm_t = xin.tile([E, f], fr, tag="gm", name="gm_t")
            nc.sync.dma_start(out=gm_t, in_=gm_d.ap()[:, n0 : n0 + f])

            # Gating chain, split into stages interleaved with expert blocks.
            gs = {}

            def g_gate():
                pgl = pmid.tile([L, f], fp32, tag="pm", name="pgl")
                nc.tensor.matmul(pgl, wg_s[:, 0, :], xg_t[:, 0, :], start=True, stop=False)
                nc.tensor.matmul(pgl, wg_s[:, 1, :], xg_t[:, 1, :], start=False, stop=True)
                # stack: [gl ; gl^2] on 128 partitions
                gk = wk.tile([128, f], fr, tag="gk", name="gk")
                nc.scalar.activation(gk[0:L, :], pgl, AF.Identity, bias=bg_b)
                nc.gpsimd.tensor_tensor(out=gk[L : 2 * L, :], in0=gk[0:L, :].bitcast(fp32),
                                        in1=gk[0:L, :].bitcast(fp32), op=OP.mult)
                gs["gk"] = gk

            def g_mlp1():
                hg = wk.tile([128, 2, f], fr, tag="hg", name="hg")
                for hc in range(2):
                    phg = pbig.tile([128, f], fp32, tag="pb", name="phg")
                    nc.tensor.matmul(phg, amat_s[:, hc, :], gs["gk"], start=True, stop=True)
                    nc.scalar.activation(hg[:, hc, :], phg, AF.Relu,
                                         bias=b128_s[:, 32 + hc : 33 + hc])
                gs["hg"] = hg

            def g_mlp2():
                p1 = pmid.tile([E, f], fp32, tag="pm", name="p1")
                nc.tensor.matmul(p1, wgm2_s[:, 0, :], gs["hg"][:, 0, :], start=True, stop=False)
                nc.tensor.matmul(p1, wgm2_s[:, 1, :], gs["hg"][:, 1, :], start=False, stop=False)
                # z = logits2 + gumbel via identity-matmul accumulation
                nc.tensor.matmul(p1, i8_s, gm_t, start=False, stop=True)
                w_t = wk.tile([E, f], fr, tag="w", name="w_t")
                nc.scalar.activation(w_t, p1, AF.Exp, bias=bgm2_b)
                gs["w"] = w_t

            stages = ([pending_tail] if pending_tail is not None else []) \
                + [g_gate, g_mlp1, g_mlp2]

            # ---- expert branch (fp8 DoubleRow), software-pipelined so the
            # PE never waits on a relu: We1 of expert e+1 is issued before
            # We2 of expert e, and We3 lags one expert behind. ----
            peo = ppeo.tile([16, f], fp32, tag="peo", name="peo")
            h1qs, h2qs = {}, {}

            def emit_we1(e):
                h1q = hp.tile([128, 2, f], f8, tag="h1", name="h1q")
                for hc in range(2):
                    ph = pbig.tile([128, f], fp32, tag="pb", name="ph1")
                    nc.tensor.matmul(ph, we1_s[:, e * 2 + hc, :, :], xs_t,
                                     start=True, stop=True, perf_mode=PM)
                    bias = b128_s[:, e * 2 + hc : e * 2 + hc + 1]
                    if hc == 0:
                        nc.scalar.activation(h1q[:, 0, :], ph, AF.Relu, bias=bias)
                    else:
                        nc.vector.tensor_scalar(out=h1q[:, 1, :], in0=ph, scalar1=bias,
                                                scalar2=0.0, op0=OP.add, op1=OP.max)
                h1qs[e] = h1q

            def emit_we2(e):
                h2q = hp.tile([128, 2, f], f8, tag="h2", name="h2q")
                for kc in range(2):
                    ph = pbig.tile([128, f], fp32, tag="pb", name="ph2")
                    nc.tensor.matmul(ph, we2_s[:, e * 2 + kc, :, :], h1qs.pop(e) if kc else h1qs[e],
                                     start=True, stop=True, perf_mode=PM)
                    bias = b128_s[:, 16 + e * 2 + kc : 17 + e * 2 + kc]
                    if kc == 0:
                        nc.vector.tensor_scalar(out=h2q[:, 0, :], in0=ph, scalar1=bias,
                                                scalar2=0.0, op0=OP.add, op1=OP.max)
                    else:
                        nc.scalar.activation(h2q[:, 1, :], ph, AF.Relu, bias=bias)
                h2qs[e] = h2q

            def emit_we3(e):
                nc.tensor.matmul(peo, we3_s[:, e, :, :], h2qs.pop(e),
                                 start=(e == 0), stop=(e == E - 1), perf_mode=PM)

            emit_we1(0)
            for e in range(E):
                if e + 1 < E:
                    emit_we1(e + 1)
                if e < len(stages):
                    stages[e]()
                emit_we2(e)
                if e >= 2:
                    emit_we3(e - 2)
            emit_we3(E - 2)
            emit_we3(E - 1)

            # Tail (ewp, den/num, writeback) is deferred into the NEXT
            # tile's interleave slots so the PE streams next-tile We1s
            # while this tile's reduction chain drains.
            def make_tail(peo=peo, w_t=gs["w"], n0=n0):
                def tail():
                    ewp_t = wk.tile([E, f], fr, tag="ewp", name="ewp_t")
                    nc.vector.scalar_tensor_tensor(out=ewp_t, in0=peo[0:E, :],
                                                   scalar=be3_b,
                                                   in1=w_t.bitcast(fp32),
                                                   op0=OP.add, op1=OP.mult)
                    pdn = pmid.tile([2, f], fp32, tag="pm", name="pdn")
                    nc.tensor.matmul(pdn, dnw_s[:, 0:2], w_t, start=True, stop=False)
                    nc.tensor.matmul(pdn, dnw_s[:, 2:4], ewp_t, start=False, stop=True)
                    dn_s = wk.tile([2, f], fp32, tag="dns", name="dn_s")
                    nc.scalar.activation(dn_s, pdn, AF.Identity)
                    nc.sync.dma_start(out=scr_d.ap()[:, n0 : n0 + f], in_=dn_s)
                return tail

            pending_tail = make_tail()
        pending_tail()

        # ---- post-pass: out = 1 / (1 + exp(-num/den)), full-width ----
        dn2 = cw.tile([PP, 2, PJ], fp32)
        nc.sync.dma_start(out=dn2, in_=scr_d.ap().rearrange("c (p j) -> p c j", p=PP))
        denr2 = cw.tile([PP, PJ], fp32)
        nc.vector.reciprocal(denr2, dn2[:, 0, :])
        rat2 = cw.tile([PP, PJ], fp32)
        nc.vector.tensor_mul(rat2, dn2[:, 1, :], denr2)
        en2 = cw.tile([PP, PJ], fp32)
        nc.scalar.activation(en2, rat2, AF.Exp, scale=-1.0)
        ep2 = cw.tile([PP, PJ], fp32)
        nc.vector.tensor_scalar_add(ep2, en2, 1.0)
        outp = cw.tile([PP, PJ], fp32)
        nc.vector.reciprocal(outp, ep2)
        nc.sync.dma_start(out=out_d.ap().rearrange("(p j) -> p j", p=PP), in_=outp)

    nc.compile()
    return nc


def _pack_weights(ins):
    """Host-side packing of parameters into SBUF-ready layouts."""
    import ml_dtypes

    f32 = np.float32
    f8 = ml_dtypes.float8_e4m3
    We1, be1 = np.asarray(ins["We1"], f32), np.asarray(ins["be1"], f32)
    We2, be2 = np.asarray(ins["We2"], f32), np.asarray(ins["be2"], f32)
    We3, be3 = np.asarray(ins["We3"], f32), np.asarray(ins["be3"], f32)
    Wg, bg = np.asarray(ins["Wg"], f32), np.asarray(ins["bg"], f32)
    centers = np.asarray(ins["centers"], f32)
    Wgm1, bgm1 = np.asarray(ins["Wgm1"], f32), np.asarray(ins["bgm1"], f32)
    Wgm2, bgm2 = np.asarray(ins["Wgm2"], f32), np.asarray(ins["bgm2"], f32)

    # expert weights, fp8, DoubleRow layout [k, e*2+chunk, ktile, m]
    we1_p = np.ascontiguousarray(
        We1.reshape(E, 2, 128, 2, 128).transpose(2, 0, 3, 1, 4)
        .reshape(128, 2 * E, 2, 128)).astype(f8)
    we2_p = np.ascontiguousarray(
        We2.reshape(E, 2, 128, 2, 128).transpose(2, 0, 3, 1, 4)
        .reshape(128, 2 * E, 2, 128)).astype(f8)
    # we1 layout check: [k, e*2+hc, i, m] = We1[e, i*128+k, hc*128+m]
    we3_p = np.zeros((128, E, 2, 16), f32)
    for e in range(E):
        for i in range(2):
            we3_p[:, e, i, e] = We3[e, i * 128 : (i + 1) * 128]
    we3_p = we3_p.astype(f8)

    wg_p = np.ascontiguousarray(Wg.reshape(2, 128, L).transpose(1, 0, 2))
    # fused distance + gating-mlp1: logits1 = gl@A1 + gl^2@A2 + beff
    A1 = 2.0 * centers.T @ Wgm1                       # [L, H]
    A2 = np.repeat(-Wgm1.sum(axis=0, keepdims=True), L, axis=0)  # [L, H]
    amat = np.zeros((128, 2, 128), f32)
    Afull = np.concatenate([A1, A2], axis=0)          # [128, H]
    amat[:, 0, :] = Afull[:, 0:128]
    amat[:, 1, :] = Afull[:, 128:256]
    beff = bgm1 - (centers * centers).sum(axis=1) @ Wgm1   # [H]

    W2c = Wgm2 - Wgm2.mean(axis=1, keepdims=True)
    wgm2_p = np.ascontiguousarray(W2c.reshape(2, 128, E).transpose(1, 0, 2))

    dnw = np.zeros((E, 4), f32)
    dnw[:, 0] = 1.0   # den from w
    dnw[:, 3] = 1.0   # num from ewp
    b128 = np.zeros((128, 34), f32)
    b128[:, 0:16] = be1.reshape(E, 2, 128).transpose(2, 0, 1).reshape(128, 16)
    b128[:, 16:32] = be2.reshape(E, 2, 128).transpose(2, 0, 1).reshape(128, 16)
    b128[:, 32:34] = beff.reshape(2, 128).T
    csm = np.zeros((L, 4), f32)
    csm[:, 0] = bg
    csm[0:E, 1] = bgm2
    csm[0:E, 2] = be3
    return {
        "we1": we1_p, "we2": we2_p, "we3": we3_p, "wg": wg_p, "amat": amat,
        "wgm2": wgm2_p, "i8": np.eye(E, dtype=f32), "dnw": dnw,
        "b128": b128, "csm": csm,
    }


_NC_CACHE = {}


def _get_nc(nc_n, f):
    key = (nc_n, f)
    if key not in _NC_CACHE:
        _NC_CACHE[key] = _build_nc(nc_n, f)
    return _NC_CACHE[key]


def kernel(**inputs) -> np.ndarray:
    import ml_dtypes

    from concourse.bass_utils import run_bass_kernel_spmd

    nc = _get_nc(NC_N, F)
    wmaps = _pack_weights(inputs)

    f32 = np.float32
    score_T8 = np.ascontiguousarray(
        np.asarray(inputs["score_input"], f32).T).astype(ml_dtypes.float8_e4m3)
    gate_T = np.ascontiguousarray(np.asarray(inputs["gate_input"], f32).T)
    gum_T = np.ascontiguousarray(np.asarray(inputs["gumbel_noise"], f32).T)

    in_maps = []
    for c in range(M_CORES):
        s = slice(c * NC_N, (c + 1) * NC_N)
        m = dict(wmaps)
        m["xs"] = np.ascontiguousarray(score_T8[:, s])
        m["xg"] = np.ascontiguousarray(gate_T[:, s])
        m["gm"] = np.ascontiguousarray(gum_T[:, s])
        in_maps.append(m)

    res = run_bass_kernel_spmd(nc, in_maps, core_ids=list(range(M_CORES)))
    out = np.concatenate([res.results[c]["out"] for c in range(M_CORES)])
    return out.reshape(N, 1).astype(np.float32)


if __name__ == "__main__":
    import jax

    with jax.default_device(jax.local_devices(backend="cpu")[0]):
        import reference

        ins = reference.setup_inputs()
        ins = {k: np.asarray(v) for k, v in ins.items()}
        expected = np.asarray(reference.reference(**ins))
    out = kernel(**ins)
    err = np.abs(out - expected).max()
    print("max abs err:", err, "rel:", err / np.abs(expected).max())
